# revision 32
# baseline (speedup 1.0000x reference)
"""Multi-head attention (b=2, n=2048, d=1024, H=16 heads) on 8 TRN2 NeuronCores.

Sharding: core c = (b, g) with b = c // 4 (data parallel over batch) and
g = c % 4 (tensor parallel over head groups of 4 heads).  Each core computes
qkv projections for its 4 heads, full softmax attention for those heads, and
a partial output projection y_partial = A_heads @ w_out[g*256:(g+1)*256].
The host sums the 4 partials per batch and adds b_out.

Per-core schedule: the kernel is paced by ScalarE (the exp of 16.8M score
elements is ~143us of ACTIVATE work, the largest single-engine budget).
The 8 (chunk, pair) attention blocks x 16 nk-tiles are flattened into one
128-iteration software pipeline emitted as
    background(i) | sc(i+1) | PV(i)         on the PE queue
    ACT(i)                                   on the Scalar queue
so the two K=64 score matmuls of iteration i+1 (concurrent in disjoint PE
row groups) and one slot's worth of background matmuls run inside ACT(i)'s
window, and the PV pair lands right at ACT(i)'s end.  All projection work
(qkv in, attention out) is diced into single-matmul background units and
spread over the pipeline with deadline-ordered slots; block order is
(c,0) x 4 then (c,1) x 4 so the pr=1 weights/projections are not needed
until halfway through.  xt is DMA'd column-chunk-major so the first score
chains unblock at ~1/4 of the transfer.  The softmax epilogue (PSUM->SBUF
copies, 1-lane reciprocal of the ones-column denominator row, gpsimd
partition-broadcast, multiply) is emitted one block behind the pipeline so
its slow DVE reciprocals stay off the PSUM-release critical path.  Output
projections trail their chunk's epilogues; the final chunk's are pipelined
against the last epilogue's split pieces.
Matmuls run in bf16 (fp32 PSUM accumulation).
"""

import os
import sys

for _p in ("/opt/trn_rl_repo",):
    if _p not in sys.path and os.path.isdir(_p):
        sys.path.insert(0, _p)

import ml_dtypes
import numpy as np

import concourse.bass as bass
import concourse.mybir as mybir
import concourse.tile as tile
from concourse import bacc

P = 128
D = 1024          # model dim
N = 2048          # sequence length
HD = 64           # head dim
GH = 4            # heads per core
DG = GH * HD      # 256 projected cols per core
KD = D // P       # 8 k-tiles over model dim
NT = N // P       # 16 tiles over sequence
QC = 512          # n_q chunk size
NQC = N // QC     # 4 chunks
SCALE = HD ** -0.5

F32 = mybir.dt.float32
BF16 = mybir.dt.bfloat16

Exp = mybir.ActivationFunctionType.Exp

# block order: all pr=0 chunks first, then pr=1 — delays the second pair's
# kt/qt deadlines to the pipeline's second half
SEQ = [(0, 0), (1, 0), (2, 0), (3, 0), (0, 1), (1, 1), (2, 1), (3, 1)]
ITERS = [(bi, c, pr, t) for bi, (c, pr) in enumerate(SEQ) for t in range(NT)]
NI = len(ITERS)  # 128


def build_nc():
    nc = bacc.Bacc("TRN2")

    xt = nc.declare_dram_parameter("xt", [D, N], BF16, isOutput=False)
    wq = nc.declare_dram_parameter("wq", [D, DG], BF16, isOutput=False)
    wk = nc.declare_dram_parameter("wk", [D, DG], BF16, isOutput=False)
    wv = nc.declare_dram_parameter("wv", [D, DG], BF16, isOutput=False)
    wo = nc.declare_dram_parameter("wo", [DG, D], BF16, isOutput=False)
    y = nc.declare_dram_parameter("y", [N, D], F32, isOutput=True)

    xt_r = xt[:, :].rearrange("(o p) n -> p o n", p=P)    # [128, 8, 2048]
    wq_r = wq[:, :].rearrange("(o p) n -> p o n", p=P)    # [128, 8, 256]
    wk_r = wk[:, :].rearrange("(o p) n -> p o n", p=P)
    wv_r = wv[:, :].rearrange("(o p) n -> p o n", p=P)
    wo_r = wo[:, :].rearrange("(o p) n -> p o n", p=P)    # [128, 2, 1024]
    y_r = y[:, :].rearrange("(o p) n -> p o n", p=P)      # [128, 16, 1024]

    with tile.TileContext(nc) as tc, nc.allow_low_precision("bf16 attention"):
        with (
            tc.tile_pool(name="wpool", bufs=1) as wpool,
            tc.tile_pool(name="qkvpool", bufs=1) as qkvpool,
            tc.tile_pool(name="attnpool", bufs=1) as attnpool,
            tc.tile_pool(name="xpool", bufs=1) as xpool,
            tc.tile_pool(name="epool", bufs=12) as epool,
            tc.tile_pool(name="work", bufs=4) as work,
            tc.tile_pool(name="outp", bufs=2) as outp,
            tc.tile_pool(name="ps_a", bufs=2, space="PSUM") as ps_a,
            tc.tile_pool(name="ps_st", bufs=2, space="PSUM") as ps_st,
            tc.tile_pool(name="ps_o", bufs=2, space="PSUM") as ps_o,
        ):
            # --- load weights (wk first: first matmuls need wk + xt k0) ---
            wk_sb = wpool.tile([P, KD, DG], BF16, tag="wk")
            nc.sync.dma_start(wk_sb[:], wk_r)
            wq_sb = wpool.tile([P, KD, DG], BF16, tag="wq")
            wv_sb = wpool.tile([P, KD, DG], BF16, tag="wv")
            wo_sb = wpool.tile([P, 2, D], BF16, tag="wo")

            # xt loaded column-chunk-major: the first kq/qt chains and early v
            # tiles only touch columns 0..511, so chunk 0 unblocks the
            # pipeline at ~1/4 of the full 4MB transfer
            xt_sb = xpool.tile([P, KD, N], BF16, tag="xt")
            cs0 = slice(0, QC)
            for k in range(KD):
                nc.sync.dma_start(xt_sb[:, k, cs0], xt_r[:, k, cs0])
            nc.sync.dma_start(wq_sb[:], wq_r)
            nc.sync.dma_start(wv_sb[:], wv_r)
            for c in range(1, NQC):
                csc = slice(c * QC, (c + 1) * QC)
                for k in range(KD):
                    nc.sync.dma_start(xt_sb[:, k, csc], xt_r[:, k, csc])
            nc.sync.dma_start(wo_sb[:], wo_r)

            # --- persistent tensors ---
            qt_sb = qkvpool.tile([P, 2, N], BF16, tag="qt")   # [256, 2048] qT
            kt_sb = qkvpool.tile([P, 2, N], BF16, tag="kt")   # [256, 2048] kT
            vg_sb = qkvpool.tile([P, NT, GH, 66], BF16, tag="vg")  # v + ones col
            nc.scalar.copy(
                vg_sb[:, :, :, HD:], nc.const_aps.tensor(1.0, (P, NT, GH, 2), F32)
            )
            at_sb = attnpool.tile([P, 2, N], BF16, tag="at")  # attn_outT [256, 2048]

            # ---------- background unit machinery ----------
            # each unit is a closure emitting ~1 matmul (or copy/dma)
            kq_state = {}
            # most recent PSUM-accumulator-releasing DVE copy: epilogue
            # reciprocals are pinned behind it (the cost model underestimates
            # 1-lane RECIPROCAL, so without the pin the static DVE order can
            # place a ~3.3us reciprocal ahead of a release copy, stalling the
            # next background chain's matmuls ~6us at block boundaries)
            last_release = [None]

            def kq_unit(which, w_sb, dst, m, c, k):
                def emit():
                    key = (which, m, c)
                    if k == 0:
                        kq_state[key] = ps_a.tile(
                            [P, QC], F32, tag="a", name=f"{which}ps_{m}_{c}"
                        )
                    ps = kq_state[key]
                    nc.tensor.matmul(
                        ps[:],
                        w_sb[:, k, m * P:(m + 1) * P],
                        xt_sb[:, k, c * QC:(c + 1) * QC],
                        start=(k == 0),
                        stop=(k == KD - 1),
                    )
                    if k == KD - 1:
                        last_release[0] = nc.vector.tensor_copy(
                            dst[:, m, c * QC:(c + 1) * QC], ps[:]
                        )
                        del kq_state[key]
                return emit

            def kq_units(which, w_sb, dst, m, c):
                return [kq_unit(which, w_sb, dst, m, c, k) for k in range(KD)]

            def v_unit(t):
                # whole v group (8 matmuls + copy) as one unit
                def emit():
                    ps = ps_a.tile([P, QC], F32, tag="a", name=f"vps_{t}")
                    for k in range(KD):
                        nc.tensor.matmul(
                            ps[:, :DG],
                            xt_sb[:, k, t * P:(t + 1) * P],
                            wv_sb[:, k, :],
                            start=(k == 0),
                            stop=(k == KD - 1),
                        )
                    last_release[0] = nc.vector.tensor_copy(
                        vg_sb[:, t, :, 0:HD],
                        ps[:, :DG].rearrange("p (h e) -> p h e", h=GH),
                    )
                return emit

            proj_state = {}

            def proj_unit(m, nn, ks):
                def emit():
                    key = (m, nn)
                    if ks == 0:
                        proj_state[key] = ps_a.tile(
                            [P, QC], F32, tag="a", name=f"yps_{m}_{nn}"
                        )
                    ps = proj_state[key]
                    nc.tensor.matmul(
                        ps[:],
                        at_sb[:, ks, m * P:(m + 1) * P],
                        wo_sb[:, ks, nn * QC:(nn + 1) * QC],
                        start=(ks == 0),
                        stop=(ks == 1),
                    )
                    if ks == 1:
                        ysb = outp.tile([P, QC], F32, tag="y", name=f"y_{m}_{nn}")
                        last_release[0] = nc.vector.tensor_copy(ysb[:], ps[:])
                        nc.sync.dma_start(y_r[:, m, nn * QC:(nn + 1) * QC], ysb[:])
                        del proj_state[key]
                return emit

            def proj_units(c):
                return [
                    proj_unit(4 * c + mi, nn, ks)
                    for mi in range(4) for nn in range(2) for ks in range(2)
                ]

            # ---------- the background schedule: slot -> [units] ----------
            bg = {i: [] for i in range(NI)}

            def spread(units, s0, s1):
                # spread units evenly over slots [s0, s1)
                ns = s1 - s0
                for j, u in enumerate(units):
                    bg[s0 + (j * ns) // len(units)].append(u)

            # v groups: one per slot, just ahead of their PV deadline
            for t in range(1, NT):
                bg[t - 1].append(v_unit(t))
            # kt pair 0 remaining chunks (c=1,2,3 due at slots 4c)
            for c in (1, 2, 3):
                spread(kq_units("k", wk_sb, kt_sb, 0, c), 4 * (c - 1), 4 * c)
            # qt (0, c) due at slot 16c
            spread(kq_units("q", wq_sb, qt_sb, 0, 1), 12, 16)
            spread(kq_units("q", wq_sb, qt_sb, 0, 2), 24, 32)
            spread(kq_units("q", wq_sb, qt_sb, 0, 3), 40, 48)
            # kt pair 1 (due from slot 64) and qt (1, 0) (due slot 64)
            for c in range(4):
                spread(kq_units("k", wk_sb, kt_sb, 1, c), 44 + 4 * c, 48 + 4 * c)
            spread(kq_units("q", wq_sb, qt_sb, 1, 0), 60, 64)
            # qt (1, c) due at slot 64 + 16c; the pr=1 qt chains and the
            # output-projection chains share the 2 PSUM accumulator slots,
            # so their slot ranges are kept DISJOINT (overlap makes a
            # 3-chain rotation whose release copies stall chain starts)
            spread(kq_units("q", wq_sb, qt_sb, 1, 1), 72, 80)
            spread(proj_units(0), 82, 92)
            spread(kq_units("q", wq_sb, qt_sb, 1, 2), 92, 96)
            spread(proj_units(1), 98, 108)
            spread(kq_units("q", wq_sb, qt_sb, 1, 3), 108, 112)
            spread(proj_units(2), 114, 126)
            # proj_units(3) run in the tail

            # ---------- attention emitters ----------
            def emit_sc(i):
                bi, c, pr, t = ITERS[i]
                cs = slice(c * QC, (c + 1) * QC)
                ts_ = slice(t * P, (t + 1) * P)
                st = ps_st.tile([P, 2, QC], F32, tag="st", name=f"st_{bi}_{t}")
                for half in range(2):
                    hs = slice(half * HD, (half + 1) * HD)
                    nc.tensor.matmul(
                        st[:, half, :],
                        kt_sb[hs, pr, ts_],
                        qt_sb[hs, pr, cs],
                        start=True,
                        stop=True,
                    )
                return st

            def emit_act(i, st):
                bi, c, pr, t = ITERS[i]
                e = epool.tile([P, 2, QC], BF16, tag="exp", name=f"e_{bi}_{t}")
                nc.scalar.activation(e[:], st[:], Exp, scale=SCALE)
                return e

            def emit_pv(i, e, o_ps):
                bi, c, pr, t = ITERS[i]
                for half in range(2):
                    h = 2 * pr + half
                    nc.tensor.matmul(
                        o_ps[half][:],
                        vg_sb[:, t, h, 0:HD + 1],
                        e[:, half, :],
                        start=(t == 0),
                        stop=(t == NT - 1),
                    )

            def alloc_o(bi):
                o_ps = []
                for half in range(2):
                    o_full = ps_o.tile(
                        [P, QC], F32, tag="o", name=f"o_{bi}_{half}"
                    )
                    o_ps.append(o_full[: HD + 1])
                return o_ps

            def emit_epilogue_copies(bi, o_ps):
                o_sbs = []
                for half in range(2):
                    o_sb = work.tile(
                        [HD + 1, QC], F32, tag="osb", name=f"osb_{bi}_{half}"
                    )
                    nc.vector.tensor_copy(o_sb[:], o_ps[half][:])
                    o_sbs.append(o_sb)
                return o_sbs

            def emit_epilogue_norm(bi, o_sbs, split=1, piece_done=None):
                _, c, pr, _ = ITERS[bi * NT]
                pin = last_release[0]
                w = QC // split
                for s in range(split):
                    ss = slice(s * w, (s + 1) * w)
                    for half in range(2):
                        o_sb = o_sbs[half]
                        rc = work.tile(
                            [1, w], F32, tag="rc", name=f"rc_{bi}_{half}_{s}"
                        )
                        r = nc.vector.reciprocal(rc[:], o_sb[HD:HD + 1, ss])
                        if pin is not None:
                            bass._add_dep_helper(
                                r.ins, pin.ins, sync=False, reason="defer recip"
                            )
                        rbs = work.tile(
                            [HD, w], F32, tag="rbs", name=f"rbs_{bi}_{half}_{s}"
                        )
                        nc.gpsimd.partition_broadcast(rbs[:], rc[:])
                        if half == 0:
                            nc.vector.tensor_mul(
                                at_sb[0:HD, pr, c * QC + s * w:c * QC + (s + 1) * w],
                                o_sb[0:HD, ss],
                                rbs[:],
                            )
                        else:
                            stg = work.tile(
                                [HD, w], BF16, tag="stg", name=f"stg_{bi}_{s}"
                            )
                            nc.vector.tensor_mul(stg[:], o_sb[0:HD, ss], rbs[:])
                            nc.sync.dma_start(
                                at_sb[HD:P, pr, c * QC + s * w:c * QC + (s + 1) * w],
                                stg[:],
                            )
                    if piece_done is not None:
                        piece_done(s)

            # ---------- prefix ----------
            for u in kq_units("k", wk_sb, kt_sb, 0, 0):
                u()
            for u in kq_units("q", wq_sb, qt_sb, 0, 0):
                u()
            v_unit(0)()

            # ---------- main pipeline ----------
            sts = {0: emit_sc(0)}
            es = {}
            o_cur = None
            o_copies = {}
            for i in range(NI):
                bi, c, pr, t = ITERS[i]
                if t == 0:
                    o_cur = alloc_o(bi)
                es[i] = emit_act(i, sts.pop(i))
                # background writes (kt/qt/v chunks) must be EMITTED before
                # their readers: Tile dataflow deps follow emission order
                for u in bg[i]:
                    u()
                if i + 1 < NI:
                    sts[i + 1] = emit_sc(i + 1)
                emit_pv(i, es.pop(i), o_cur)
                if t == NT - 1:
                    o_copies[bi] = emit_epilogue_copies(bi, o_cur)
                if t == 1 and bi >= 1 and (bi - 1) in o_copies:
                    # the previous block's normalization chain is emitted
                    # here (one block behind) so its slow DVE ops don't
                    # delay PSUM-releasing copies
                    emit_epilogue_norm(bi - 1, o_copies.pop(bi - 1))

            # ---------- tail: last epilogue pipelined with chunk-3 proj ----
            pu3 = proj_units(3)

            def tail_piece(s):
                # after split piece s (256 cols = m-tiles 12+2s,13+2s) is
                # normalized, its four projection units can go
                for u in pu3[8 * s:8 * s + 8]:
                    u()

            emit_epilogue_norm(
                len(SEQ) - 1, o_copies.pop(len(SEQ) - 1), split=2,
                piece_done=tail_piece,
            )

    nc.finalize()
    return nc


_NC = None


def _get_nc():
    global _NC
    if _NC is None:
        _NC = build_nc()
    return _NC


def _in_maps(x, w_qkv, w_out):
    bf = ml_dtypes.bfloat16
    x = np.asarray(x, dtype=np.float32)
    w_qkv = np.asarray(w_qkv, dtype=np.float32)
    w_out = np.asarray(w_out, dtype=np.float32)
    xts = [np.ascontiguousarray(x[b].T).astype(bf) for b in range(2)]
    wq_g = [np.ascontiguousarray(w_qkv[:, 0 * D + g * DG:0 * D + (g + 1) * DG]).astype(bf) for g in range(4)]
    wk_g = [np.ascontiguousarray(w_qkv[:, 1 * D + g * DG:1 * D + (g + 1) * DG]).astype(bf) for g in range(4)]
    wv_g = [np.ascontiguousarray(w_qkv[:, 2 * D + g * DG:2 * D + (g + 1) * DG]).astype(bf) for g in range(4)]
    wo_g = [np.ascontiguousarray(w_out[g * DG:(g + 1) * DG, :]).astype(bf) for g in range(4)]
    maps = []
    for c in range(8):
        b, g = c // 4, c % 4
        maps.append({
            "xt": xts[b],
            "wq": wq_g[g],
            "wk": wk_g[g],
            "wv": wv_g[g],
            "wo": wo_g[g],
        })
    return maps


LAST_RESULT = None


def kernel(x, w_qkv, w_out, b_out):
    from concourse.bass_utils import run_bass_kernel_spmd

    nc = _get_nc()
    maps = _in_maps(x, w_qkv, w_out)
    res = run_bass_kernel_spmd(nc, maps, list(range(8)))
    global LAST_RESULT
    LAST_RESULT = res
    out = np.zeros((2, N, D), dtype=np.float32)
    for c in range(8):
        out[c // 4] += res.results[c]["y"]
    out += np.asarray(b_out, dtype=np.float32)[None, None, :]
    return out


# revision 34
# speedup vs baseline: 1.0318x; 1.0318x over previous
"""Multi-head attention (b=2, n=2048, d=1024, H=16 heads) on 8 TRN2 NeuronCores.

Sharding: core c = (b, g) with b = c // 4 (data parallel over batch) and
g = c % 4 (tensor parallel over head groups of 4 heads).  Each core computes
qkv projections for its 4 heads, full softmax attention for those heads, and
a partial output projection y_partial = A_heads @ w_out[g*256:(g+1)*256].
The host sums the 4 partials per batch and adds b_out.

Per-core schedule: the kernel is paced by ScalarE (the exp of 16.8M score
elements is ~143us of ACTIVATE work, the largest single-engine budget).
The 8 (chunk, pair) attention blocks x 16 nk-tiles are flattened into one
128-iteration software pipeline emitted as
    background(i) | sc(i+1) | PV(i)         on the PE queue
    ACT(i)                                   on the Scalar queue
so the two K=64 score matmuls of iteration i+1 (concurrent in disjoint PE
row groups) and one slot's worth of background matmuls run inside ACT(i)'s
window, and the PV pair lands right at ACT(i)'s end.  All projection work
(qkv in, attention out) is diced into single-matmul background units and
spread over the pipeline with deadline-ordered slots; block order is
(c,0) x 4 then (c,1) x 4 so the pr=1 weights/projections are not needed
until halfway through.  xt is DMA'd column-chunk-major so the first score
chains unblock at ~1/4 of the transfer.  The softmax epilogue (PSUM->SBUF
copies, 1-lane reciprocal of the ones-column denominator row, gpsimd
partition-broadcast, multiply) is emitted one block behind the pipeline so
its slow DVE reciprocals stay off the PSUM-release critical path.  Output
projections trail their chunk's epilogues; the final chunk's are pipelined
against the last epilogue's split pieces.
Matmuls run in bf16 (fp32 PSUM accumulation).
"""

import os
import sys

for _p in ("/opt/trn_rl_repo",):
    if _p not in sys.path and os.path.isdir(_p):
        sys.path.insert(0, _p)

import ml_dtypes
import numpy as np

import concourse.bass as bass
import concourse.mybir as mybir
import concourse.tile as tile
from concourse import bacc

P = 128
D = 1024          # model dim
N = 2048          # sequence length
HD = 64           # head dim
GH = 4            # heads per core
DG = GH * HD      # 256 projected cols per core
KD = D // P       # 8 k-tiles over model dim
NT = N // P       # 16 tiles over sequence
QC = 512          # n_q chunk size
NQC = N // QC     # 4 chunks
SCALE = HD ** -0.5

F32 = mybir.dt.float32
BF16 = mybir.dt.bfloat16

Exp = mybir.ActivationFunctionType.Exp

# block order: all pr=0 chunks first, then pr=1 — delays the second pair's
# kt/qt deadlines to the pipeline's second half
SEQ = [(0, 0), (1, 0), (2, 0), (3, 0), (0, 1), (1, 1), (2, 1), (3, 1)]
ITERS = [(bi, c, pr, t) for bi, (c, pr) in enumerate(SEQ) for t in range(NT)]
NI = len(ITERS)  # 128


def build_nc():
    nc = bacc.Bacc("TRN2")

    xt = nc.declare_dram_parameter("xt", [D, N], BF16, isOutput=False)
    wq = nc.declare_dram_parameter("wq", [D, DG], BF16, isOutput=False)
    wk = nc.declare_dram_parameter("wk", [D, DG], BF16, isOutput=False)
    wv = nc.declare_dram_parameter("wv", [D, DG], BF16, isOutput=False)
    wo = nc.declare_dram_parameter("wo", [DG, D], BF16, isOutput=False)
    y = nc.declare_dram_parameter("y", [N, D], F32, isOutput=True)

    xt_r = xt[:, :].rearrange("(o p) n -> p o n", p=P)    # [128, 8, 2048]
    wq_r = wq[:, :].rearrange("(o p) n -> p o n", p=P)    # [128, 8, 256]
    wk_r = wk[:, :].rearrange("(o p) n -> p o n", p=P)
    wv_r = wv[:, :].rearrange("(o p) n -> p o n", p=P)
    wo_r = wo[:, :].rearrange("(o p) n -> p o n", p=P)    # [128, 2, 1024]
    y_r = y[:, :].rearrange("(o p) n -> p o n", p=P)      # [128, 16, 1024]

    with tile.TileContext(nc) as tc, nc.allow_low_precision("bf16 attention"):
        with (
            tc.tile_pool(name="wpool", bufs=1) as wpool,
            tc.tile_pool(name="qkvpool", bufs=1) as qkvpool,
            tc.tile_pool(name="attnpool", bufs=1) as attnpool,
            tc.tile_pool(name="xpool", bufs=1) as xpool,
            tc.tile_pool(name="epool", bufs=12) as epool,
            tc.tile_pool(name="work", bufs=4) as work,
            tc.tile_pool(name="outp", bufs=2) as outp,
            tc.tile_pool(name="ps_a", bufs=2, space="PSUM") as ps_a,
            tc.tile_pool(name="ps_st", bufs=2, space="PSUM") as ps_st,
            tc.tile_pool(name="ps_o", bufs=2, space="PSUM") as ps_o,
        ):
            # --- load weights (wk first: first matmuls need wk + xt k0) ---
            wk_sb = wpool.tile([P, KD, DG], BF16, tag="wk")
            nc.sync.dma_start(wk_sb[:], wk_r)
            wq_sb = wpool.tile([P, KD, DG], BF16, tag="wq")
            wv_sb = wpool.tile([P, KD, DG], BF16, tag="wv")
            wo_sb = wpool.tile([P, 2, D], BF16, tag="wo")

            # xt loaded column-chunk-major: the first kq/qt chains and early v
            # tiles only touch columns 0..511, so chunk 0 unblocks the
            # pipeline at ~1/4 of the full 4MB transfer
            # wq issued BEFORE the xt slabs: the qt(0,0) chain then
            # interleaves with the kt(0,0) chain as slabs arrive instead of
            # waiting ~7us for wq behind all 8 slab DMAs (issue-serialized)
            nc.sync.dma_start(wq_sb[:], wq_r)
            xt_sb = xpool.tile([P, KD, N], BF16, tag="xt")
            cs0 = slice(0, QC)
            for k in range(KD):
                nc.sync.dma_start(xt_sb[:, k, cs0], xt_r[:, k, cs0])
            nc.sync.dma_start(wv_sb[:], wv_r)
            for c in range(1, NQC):
                csc = slice(c * QC, (c + 1) * QC)
                for k in range(KD):
                    nc.sync.dma_start(xt_sb[:, k, csc], xt_r[:, k, csc])
            nc.sync.dma_start(wo_sb[:], wo_r)

            # --- persistent tensors ---
            qt_sb = qkvpool.tile([P, 2, N], BF16, tag="qt")   # [256, 2048] qT
            kt_sb = qkvpool.tile([P, 2, N], BF16, tag="kt")   # [256, 2048] kT
            vg_sb = qkvpool.tile([P, NT, GH, 66], BF16, tag="vg")  # v + ones col
            nc.scalar.copy(
                vg_sb[:, :, :, HD:], nc.const_aps.tensor(1.0, (P, NT, GH, 2), F32)
            )
            at_sb = attnpool.tile([P, 2, N], BF16, tag="at")  # attn_outT [256, 2048]

            # ---------- background unit machinery ----------
            # each unit is a closure emitting ~1 matmul (or copy/dma)
            kq_state = {}
            # most recent PSUM-accumulator-releasing DVE copy: epilogue
            # reciprocals are pinned behind it (the cost model underestimates
            # 1-lane RECIPROCAL, so without the pin the static DVE order can
            # place a ~3.3us reciprocal ahead of a release copy, stalling the
            # next background chain's matmuls ~6us at block boundaries)
            last_release = [None]

            def kq_unit(which, w_sb, dst, m, c, k):
                def emit():
                    key = (which, m, c)
                    if k == 0:
                        kq_state[key] = ps_a.tile(
                            [P, QC], F32, tag="a", name=f"{which}ps_{m}_{c}"
                        )
                    ps = kq_state[key]
                    nc.tensor.matmul(
                        ps[:],
                        w_sb[:, k, m * P:(m + 1) * P],
                        xt_sb[:, k, c * QC:(c + 1) * QC],
                        start=(k == 0),
                        stop=(k == KD - 1),
                    )
                    if k == KD - 1:
                        last_release[0] = nc.vector.tensor_copy(
                            dst[:, m, c * QC:(c + 1) * QC], ps[:]
                        )
                        del kq_state[key]
                return emit

            def kq_units(which, w_sb, dst, m, c):
                return [kq_unit(which, w_sb, dst, m, c, k) for k in range(KD)]

            def v_unit(t):
                # whole v group (8 matmuls + copy) as one unit
                def emit():
                    ps = ps_a.tile([P, QC], F32, tag="a", name=f"vps_{t}")
                    for k in range(KD):
                        nc.tensor.matmul(
                            ps[:, :DG],
                            xt_sb[:, k, t * P:(t + 1) * P],
                            wv_sb[:, k, :],
                            start=(k == 0),
                            stop=(k == KD - 1),
                        )
                    last_release[0] = nc.vector.tensor_copy(
                        vg_sb[:, t, :, 0:HD],
                        ps[:, :DG].rearrange("p (h e) -> p h e", h=GH),
                    )
                return emit

            proj_state = {}

            def proj_unit(m, nn, ks):
                def emit():
                    key = (m, nn)
                    if ks == 0:
                        proj_state[key] = ps_a.tile(
                            [P, QC], F32, tag="a", name=f"yps_{m}_{nn}"
                        )
                    ps = proj_state[key]
                    nc.tensor.matmul(
                        ps[:],
                        at_sb[:, ks, m * P:(m + 1) * P],
                        wo_sb[:, ks, nn * QC:(nn + 1) * QC],
                        start=(ks == 0),
                        stop=(ks == 1),
                    )
                    if ks == 1:
                        ysb = outp.tile([P, QC], F32, tag="y", name=f"y_{m}_{nn}")
                        last_release[0] = nc.vector.tensor_copy(ysb[:], ps[:])
                        nc.sync.dma_start(y_r[:, m, nn * QC:(nn + 1) * QC], ysb[:])
                        del proj_state[key]
                return emit

            def proj_units(c):
                return [
                    proj_unit(4 * c + mi, nn, ks)
                    for mi in range(4) for nn in range(2) for ks in range(2)
                ]

            # ---------- the background schedule: slot -> [units] ----------
            bg = {i: [] for i in range(NI)}

            def spread(units, s0, s1):
                # spread units evenly over slots [s0, s1)
                ns = s1 - s0
                for j, u in enumerate(units):
                    bg[s0 + (j * ns) // len(units)].append(u)

            # v groups: one per slot, just ahead of their PV deadline
            for t in range(1, NT):
                bg[t - 1].append(v_unit(t))
            # kt pair 0 remaining chunks (c=1,2,3 due at slots 4c)
            for c in (1, 2, 3):
                spread(kq_units("k", wk_sb, kt_sb, 0, c), 4 * (c - 1), 4 * c)
            # qt (0, c) due at slot 16c
            spread(kq_units("q", wq_sb, qt_sb, 0, 1), 12, 16)
            spread(kq_units("q", wq_sb, qt_sb, 0, 2), 24, 32)
            spread(kq_units("q", wq_sb, qt_sb, 0, 3), 40, 48)
            # kt pair 1 (due from slot 64) and qt (1, 0) (due slot 64)
            for c in range(4):
                spread(kq_units("k", wk_sb, kt_sb, 1, c), 44 + 4 * c, 48 + 4 * c)
            spread(kq_units("q", wq_sb, qt_sb, 1, 0), 60, 64)
            # qt (1, c) due at slot 64 + 16c
            spread(kq_units("q", wq_sb, qt_sb, 1, 1), 72, 80)
            spread(kq_units("q", wq_sb, qt_sb, 1, 2), 88, 96)
            spread(kq_units("q", wq_sb, qt_sb, 1, 3), 104, 112)
            # output projection: chunk c ready after epilogue of (c, 1) which
            # trails block 4+c by ~2 slots
            spread(proj_units(0), 86, 102)
            spread(proj_units(1), 102, 118)
            spread(proj_units(2), 118, 128)
            # proj_units(3) run in the tail

            # ---------- attention emitters ----------
            def emit_sc(i):
                bi, c, pr, t = ITERS[i]
                cs = slice(c * QC, (c + 1) * QC)
                ts_ = slice(t * P, (t + 1) * P)
                st = ps_st.tile([P, 2, QC], F32, tag="st", name=f"st_{bi}_{t}")
                for half in range(2):
                    hs = slice(half * HD, (half + 1) * HD)
                    nc.tensor.matmul(
                        st[:, half, :],
                        kt_sb[hs, pr, ts_],
                        qt_sb[hs, pr, cs],
                        start=True,
                        stop=True,
                    )
                return st

            def emit_act(i, st):
                bi, c, pr, t = ITERS[i]
                e = epool.tile([P, 2, QC], BF16, tag="exp", name=f"e_{bi}_{t}")
                nc.scalar.activation(e[:], st[:], Exp, scale=SCALE)
                return e

            def emit_pv(i, e, o_ps):
                bi, c, pr, t = ITERS[i]
                for half in range(2):
                    h = 2 * pr + half
                    nc.tensor.matmul(
                        o_ps[half][:],
                        vg_sb[:, t, h, 0:HD + 1],
                        e[:, half, :],
                        start=(t == 0),
                        stop=(t == NT - 1),
                    )

            def alloc_o(bi):
                o_ps = []
                for half in range(2):
                    o_full = ps_o.tile(
                        [P, QC], F32, tag="o", name=f"o_{bi}_{half}"
                    )
                    o_ps.append(o_full[: HD + 1])
                return o_ps

            def emit_epilogue_copies(bi, o_ps):
                o_sbs = []
                for half in range(2):
                    o_sb = work.tile(
                        [HD + 1, QC], F32, tag="osb", name=f"osb_{bi}_{half}"
                    )
                    nc.vector.tensor_copy(o_sb[:], o_ps[half][:])
                    o_sbs.append(o_sb)
                return o_sbs

            def emit_epilogue_norm(bi, o_sbs, split=1, piece_done=None):
                _, c, pr, _ = ITERS[bi * NT]
                pin = last_release[0]
                w = QC // split
                for s in range(split):
                    ss = slice(s * w, (s + 1) * w)
                    for half in range(2):
                        o_sb = o_sbs[half]
                        rc = work.tile(
                            [1, w], F32, tag="rc", name=f"rc_{bi}_{half}_{s}"
                        )
                        r = nc.vector.reciprocal(rc[:], o_sb[HD:HD + 1, ss])
                        if pin is not None:
                            bass._add_dep_helper(
                                r.ins, pin.ins, sync=False, reason="defer recip"
                            )
                        rbs = work.tile(
                            [HD, w], F32, tag="rbs", name=f"rbs_{bi}_{half}_{s}"
                        )
                        nc.gpsimd.partition_broadcast(rbs[:], rc[:])
                        if half == 0:
                            nc.vector.tensor_mul(
                                at_sb[0:HD, pr, c * QC + s * w:c * QC + (s + 1) * w],
                                o_sb[0:HD, ss],
                                rbs[:],
                            )
                        else:
                            stg = work.tile(
                                [HD, w], BF16, tag="stg", name=f"stg_{bi}_{s}"
                            )
                            nc.vector.tensor_mul(stg[:], o_sb[0:HD, ss], rbs[:])
                            nc.sync.dma_start(
                                at_sb[HD:P, pr, c * QC + s * w:c * QC + (s + 1) * w],
                                stg[:],
                            )
                    if piece_done is not None:
                        piece_done(s)

            # ---------- prefix ----------
            for u in kq_units("k", wk_sb, kt_sb, 0, 0):
                u()
            for u in kq_units("q", wq_sb, qt_sb, 0, 0):
                u()
            v_unit(0)()

            # ---------- main pipeline ----------
            sts = {0: emit_sc(0)}
            es = {}
            o_cur = None
            o_copies = {}
            for i in range(NI):
                bi, c, pr, t = ITERS[i]
                if t == 0:
                    o_cur = alloc_o(bi)
                es[i] = emit_act(i, sts.pop(i))
                # background writes (kt/qt/v chunks) must be EMITTED before
                # their readers: Tile dataflow deps follow emission order
                for u in bg[i]:
                    u()
                if i + 1 < NI:
                    sts[i + 1] = emit_sc(i + 1)
                emit_pv(i, es.pop(i), o_cur)
                if t == NT - 1:
                    o_copies[bi] = emit_epilogue_copies(bi, o_cur)
                if t == 1 and bi >= 1 and (bi - 1) in o_copies:
                    # the previous block's normalization chain is emitted
                    # here (one block behind) so its slow DVE ops don't
                    # delay PSUM-releasing copies
                    emit_epilogue_norm(bi - 1, o_copies.pop(bi - 1))

            # ---------- tail: last epilogue pipelined with chunk-3 proj ----
            pu3 = proj_units(3)

            def tail_piece(s):
                # after split piece s (256 cols = m-tiles 12+2s,13+2s) is
                # normalized, its four projection units can go
                for u in pu3[8 * s:8 * s + 8]:
                    u()

            emit_epilogue_norm(
                len(SEQ) - 1, o_copies.pop(len(SEQ) - 1), split=2,
                piece_done=tail_piece,
            )

    nc.finalize()
    return nc


_NC = None


def _get_nc():
    global _NC
    if _NC is None:
        _NC = build_nc()
    return _NC


def _in_maps(x, w_qkv, w_out):
    bf = ml_dtypes.bfloat16
    x = np.asarray(x, dtype=np.float32)
    w_qkv = np.asarray(w_qkv, dtype=np.float32)
    w_out = np.asarray(w_out, dtype=np.float32)
    xts = [np.ascontiguousarray(x[b].T).astype(bf) for b in range(2)]
    wq_g = [np.ascontiguousarray(w_qkv[:, 0 * D + g * DG:0 * D + (g + 1) * DG]).astype(bf) for g in range(4)]
    wk_g = [np.ascontiguousarray(w_qkv[:, 1 * D + g * DG:1 * D + (g + 1) * DG]).astype(bf) for g in range(4)]
    wv_g = [np.ascontiguousarray(w_qkv[:, 2 * D + g * DG:2 * D + (g + 1) * DG]).astype(bf) for g in range(4)]
    wo_g = [np.ascontiguousarray(w_out[g * DG:(g + 1) * DG, :]).astype(bf) for g in range(4)]
    maps = []
    for c in range(8):
        b, g = c // 4, c % 4
        maps.append({
            "xt": xts[b],
            "wq": wq_g[g],
            "wk": wk_g[g],
            "wv": wv_g[g],
            "wo": wo_g[g],
        })
    return maps


LAST_RESULT = None


def kernel(x, w_qkv, w_out, b_out):
    from concourse.bass_utils import run_bass_kernel_spmd

    nc = _get_nc()
    maps = _in_maps(x, w_qkv, w_out)
    res = run_bass_kernel_spmd(nc, maps, list(range(8)))
    global LAST_RESULT
    LAST_RESULT = res
    out = np.zeros((2, N, D), dtype=np.float32)
    for c in range(8):
        out[c // 4] += res.results[c]["y"]
    out += np.asarray(b_out, dtype=np.float32)[None, None, :]
    return out
